# revision 33
# baseline (speedup 1.0000x reference)
"""GAT 2-layer network (PyG GATConv x2 + log_softmax) on 8 Trainium2 NeuronCores.

Strategy (dst-sharded message passing):
  - Nodes are 1D-sharded across the 8 cores (6250 nodes/core). Edges (with
    self-loops) are assigned to the core owning their *destination* node.
  - Phase 0 (per core): h = x @ W1.T for the core's node slice, plus the
    per-node attention scalars asc = (h . a_src), adc = (h . a_dst), computed
    with TensorE matmuls.  Rows [h | asc] are written to a DRAM table in bf16.
  - AllGather the node table so every core has all nodes' rows.
  - Edge phase (per core): edges sorted by dst, grouped into 128-dst-node
    "dst-blocks", each padded to a uniform number B of 128-edge blocks.
    For each chunk of dst-blocks:
      * indirect-DMA gather of [h|asc] rows by src index (edge-major tiles)
      * indirect-DMA gather of adc rows by (local) dst index
      * p = exp(leaky_relu(asc_src + adc_dst))  (softmax max-subtraction is
        skipped; mathematically identical, values are small)
      * one-hot selection matrix S[e, d] = (dstslot[e] == d) built on DVE
      * PSUM-accumulated matmuls  S.T @ [h*p | p]  give, per dst node, both
        sum_e p_e * h_src_e  and the softmax denominator sum_e p_e.
      * drain: out = (sum p h)/denom + b, relu -> layer-2 rows h2 = r1 @ W2.T
        (+ fused asc2/adc2 via extra columns of W2) written to a second table.
  - AllGather the layer-2 table, run the same edge phase with 32 features and
    1 head, finish with log_softmax per node, write the output slice.

The kernel() entry point takes the FULL inputs and returns the FULL output.
"""

import math
from contextlib import ExitStack
from dataclasses import dataclass

import numpy as np

import concourse.bass as bass
import concourse.mybir as mybir
import concourse.tile as tile
from concourse import bacc
from concourse.tile_rust import add_dep_helper
from concourse.bass_utils import run_bass_kernel_spmd

F32 = mybir.dt.float32
BF16 = mybir.dt.bfloat16
I32 = mybir.dt.int32
AX = mybir.AxisListType
OP = mybir.AluOpType
AF = mybir.ActivationFunctionType

SENT_ASC = -200.0  # sentinel asc value: exp(lrelu(-200+adc)) ~= e^-40 ~ 4e-18


@dataclass(frozen=True)
class Cfg:
    N: int = 50000
    F_IN: int = 256
    HID: int = 32
    HEADS: int = 4
    CLASSES: int = 32
    NC: int = 8
    TILE_N: int = 512   # phase-0 node tile
    CHUNK_DB: int = 4   # dst-blocks per gather chunk
    B: int = 18         # uniform 128-edge blocks per dst-block (data dependent)
    GB: int = 1         # gather columns per indirect DMA (HW supports only 1)

    @property
    def F(self):  # layer-1 feature width
        return self.HID * self.HEADS

    @property
    def NPC(self):
        return self.N // self.NC

    @property
    def T0(self):
        return math.ceil(self.NPC / self.TILE_N)

    @property
    def SLICE(self):
        return self.T0 * self.TILE_N

    @property
    def NB(self):  # dst-blocks per core
        return math.ceil(self.NPC / 128)

    @property
    def SLICEP(self):  # per-core table slice incl per-core sentinel row
        return self.SLICE + 1

    @property
    def NTAB(self):  # node-table rows (NC slices, each with sentinel)
        return self.NC * self.SLICEP

    @property
    def NCH(self):  # gather chunks per core
        return math.ceil(self.NB / self.CHUNK_DB)

    @property
    def FA(self):  # layer-1 table row width: h(F) + asc(HEADS)
        return self.F + self.HEADS

    @property
    def FR(self):  # layer-1 rhs width: h*p(F) + p(HEADS)
        return self.F + self.HEADS

    @property
    def FA2(self):  # layer-2 table row width: h2(CLASSES) + asc2(1)
        return self.CLASSES + 1

    @property
    def FR2(self):  # layer-2 rhs width
        return self.CLASSES + 1


def compute_B(dst, cfg: Cfg):
    """Max 128-edge blocks needed by any (core, dst-block)."""
    NPC, NB = cfg.NPC, cfg.NB
    core = dst // NPC
    blk = (dst % NPC) // 128
    cnt = np.bincount(core * NB + blk, minlength=cfg.NC * NB)
    return int(np.max((cnt + 127) // 128))


def build_layout(src, dst, cfg: Cfg):
    """Per-core index arrays, laid out as [128, NB*B] (partition-major)."""
    NC, NPC, NB, B = cfg.NC, cfg.NPC, cfg.NB, cfg.B
    SLICEP = cfg.SLICEP
    NBB = NB * B
    SENT_ROW = cfg.SLICE  # core 0's sentinel row

    core = dst // NPC
    local = dst % NPC
    blk = local // 128
    seg = core * NB + blk
    order = np.argsort(seg, kind="stable")
    seg_s = seg[order]
    src_s = src[order]
    local_s = local[order]

    cnt = np.bincount(seg, minlength=NC * NB)
    seg_start = np.zeros(NC * NB + 1, dtype=np.int64)
    np.cumsum(cnt, out=seg_start[1:])
    pos = np.arange(len(dst)) - seg_start[seg_s]
    assert pos.max() < B * 128, "B too small for this edge distribution"

    # target coordinates inside the per-core [128, NB*B] arrays
    c_s = seg_s // NB
    blk_s = seg_s % NB
    p_s = pos % 128
    j_s = blk_s * B + pos // 128

    src_tab = np.full((NC, 128, NBB), SENT_ROW, dtype=np.int32)
    adc_idx = np.full((NC, 128, NBB), cfg.SLICE, dtype=np.int32)  # zero sentinel row
    dsl = np.zeros((NC, 128, NBB), dtype=np.float32)

    src_core = src_s // NPC
    src_local = src_s % NPC
    src_coord = (src_core * SLICEP + src_local).astype(np.int32)

    src_tab[c_s, p_s, j_s] = src_coord
    adc_idx[c_s, p_s, j_s] = local_s.astype(np.int32)
    dsl[c_s, p_s, j_s] = (local_s - blk_s * 128).astype(np.float32)
    return src_tab, adc_idx, dsl


def pack_consts(W1, a_src1, a_dst1, b1, W2, a_src2, a_dst2, b2, cfg: Cfg):
    F, H, HID, C, F_IN = cfg.F, cfg.HEADS, cfg.HID, cfg.CLASSES, cfg.F_IN
    KT = F_IN // 128
    consts = {}
    # W1T[p, i, f] = W1[f, i*128+p]
    w1t = W1.T.reshape(KT, 128, F).transpose(1, 0, 2)
    consts["w1t"] = w1t.astype(np.float32)  # cast to bf16 happens at DMA? no: stored bf16 below
    # a blockdiag: [F, 2H]: col j<H -> a_src1[j], col j>=H -> a_dst1[j-H]
    ablk = np.zeros((F, 2 * H), dtype=np.float32)
    for h in range(H):
        ablk[h * HID:(h + 1) * HID, h] = a_src1[h]
        ablk[h * HID:(h + 1) * HID, H + h] = a_dst1[h]
    consts["ablk"] = ablk
    # W2ext [F, C+2]: W2.T | (a_src2@W2).T | (a_dst2@W2).T
    w2e = np.concatenate(
        [W2.T.astype(np.float64),
         (a_src2 @ W2).T.astype(np.float64),
         (a_dst2 @ W2).T.astype(np.float64)], axis=1)
    consts["w2e"] = w2e.astype(np.float32)
    consts["rconst"] = np.tile(np.arange(128, dtype=np.float32)[None, :], (128, 1))
    consts["ident"] = np.eye(128, dtype=np.float32)
    consts["b1rep"] = np.tile(b1[None, :].astype(np.float32), (128, 1))
    consts["b2rep"] = np.tile(b2[None, :].astype(np.float32), (128, 1))
    sent1 = np.zeros((1, cfg.FA), dtype=np.float32)
    sent1[0, F:] = SENT_ASC
    consts["sent1"] = sent1
    sent2 = np.zeros((1, cfg.FA2), dtype=np.float32)
    sent2[0, C:] = SENT_ASC
    consts["sent2"] = sent2
    return consts


def build_in_maps(x, edge_index, W1, a_src1, a_dst1, b1, W2, a_src2, a_dst2, b2,
                  cfg: Cfg):
    N, NC, NPC, SLICE = cfg.N, cfg.NC, cfg.NPC, cfg.SLICE
    loops = np.arange(N, dtype=edge_index.dtype)
    src = np.concatenate([np.asarray(edge_index[0]), loops]).astype(np.int64)
    dst = np.concatenate([np.asarray(edge_index[1]), loops]).astype(np.int64)

    src_tab, adc_idx, dsl = build_layout(src, dst, cfg)
    consts = pack_consts(np.asarray(W1), np.asarray(a_src1), np.asarray(a_dst1),
                         np.asarray(b1), np.asarray(W2), np.asarray(a_src2),
                         np.asarray(a_dst2), np.asarray(b2), cfg)

    xT = np.zeros((cfg.F_IN, NC * SLICE), dtype=np.float32)
    xv = np.asarray(x).T  # [F_IN, N]
    for c in range(NC):
        xT[:, c * SLICE:c * SLICE + NPC] = xv[:, c * NPC:(c + 1) * NPC]

    bf = np.dtype("bfloat16") if False else None  # bf16 handled via ml_dtypes below
    import ml_dtypes
    to_bf16 = lambda a: a.astype(ml_dtypes.bfloat16)

    in_maps = []
    for c in range(NC):
        m = {
            "xt": to_bf16(xT[:, c * SLICE:(c + 1) * SLICE]),
            "src_tab": src_tab[c],
            "adc_idx": adc_idx[c],
            "dsl": dsl[c],
            "w1t": to_bf16(consts["w1t"]),
            "ablk": to_bf16(consts["ablk"]),
            "w2e": to_bf16(consts["w2e"]),
            "rconst": consts["rconst"],
            "ident": to_bf16(consts["ident"]),
            "b1rep": consts["b1rep"],
            "b2rep": consts["b2rep"],
            "sent1": to_bf16(consts["sent1"]),
            "sent2": to_bf16(consts["sent2"]),
        }
        in_maps.append(m)
    return in_maps


def _delay_chain(nc, tc, ctx, n):
    if not n:
        return None
    pool = ctx.enter_context(tc.tile_pool(name=f"dly{nc.next_id()}", bufs=1))
    t = pool.tile([128, 8192], F32, name=f"dlyt{nc.next_id()}")
    last = nc.vector.memset(t[:], 0.0)
    for _ in range(n):
        last = nc.vector.tensor_scalar_add(t[:], t[:], 1.0)
    return last


def build_nc(cfg: Cfg, debug_taps: bool = False, ag_delay: int = 0):
    """Build + compile the SPMD program (identical for all cores)."""
    N, NC, NPC, NB, B, SLICE, T0 = (cfg.N, cfg.NC, cfg.NPC, cfg.NB, cfg.B,
                                    cfg.SLICE, cfg.T0)
    F, H, C, F_IN = cfg.F, cfg.HEADS, cfg.CLASSES, cfg.F_IN
    FA, FR, FA2, FR2 = cfg.FA, cfg.FR, cfg.FA2, cfg.FR2
    TILE_N, CHUNK_DB, NCH = cfg.TILE_N, cfg.CHUNK_DB, cfg.NCH
    NBB = NB * B
    KT = F_IN // 128
    NTAB = cfg.NTAB
    LASTV = NPC - (NB - 1) * 128  # valid rows in last dst-block

    nc = bacc.Bacc("TRN2", target_bir_lowering=False, debug=False,
                   num_devices=NC)

    dt = nc.dram_tensor
    xt_d = dt("xt", [F_IN, SLICE], BF16, kind="ExternalInput").ap()
    src_tab_d = dt("src_tab", [128, NBB], I32, kind="ExternalInput").ap()
    adc_idx_d = dt("adc_idx", [128, NBB], I32, kind="ExternalInput").ap()
    dsl_d = dt("dsl", [128, NBB], F32, kind="ExternalInput").ap()
    w1t_d = dt("w1t", [128, KT, F], BF16, kind="ExternalInput").ap()
    ablk_d = dt("ablk", [F, 2 * H], BF16, kind="ExternalInput").ap()
    w2e_d = dt("w2e", [F, C + 2], BF16, kind="ExternalInput").ap()
    rconst_d = dt("rconst", [128, 128], F32, kind="ExternalInput").ap()
    ident_d = dt("ident", [128, 128], BF16, kind="ExternalInput").ap()
    b1rep_d = dt("b1rep", [128, F], F32, kind="ExternalInput").ap()
    b2rep_d = dt("b2rep", [128, C], F32, kind="ExternalInput").ap()
    sent1_d = dt("sent1", [1, FA], BF16, kind="ExternalInput").ap()
    sent2_d = dt("sent2", [1, FA2], BF16, kind="ExternalInput").ap()
    out_d = dt("out", [NPC, C], F32, kind="ExternalOutput").ap()
    if debug_taps:
        dbg_hext = dt("dbg_hext", [NTAB, FA], BF16, kind="ExternalOutput").ap()
        dbg_adc1 = dt("dbg_adc1", [SLICE + 1, H], BF16, kind="ExternalOutput").ap()
        dbg_h2 = dt("dbg_h2", [SLICE + 1, FA2], BF16, kind="ExternalOutput").ap()
        dbg_gbuf = dt("dbg_gbuf", [128, CHUNK_DB * B, FA], BF16, kind="ExternalOutput").ap()
        dbg_adcb = dt("dbg_adcb", [128, CHUNK_DB * B, H], BF16, kind="ExternalOutput").ap()
        dbg_pt = dt("dbg_pt", [128, CHUNK_DB * B, H], BF16, kind="ExternalOutput").ap()
        dbg_S = dt("dbg_S", [128, CHUNK_DB * B, 128], BF16, kind="ExternalOutput").ap()

    rg = [list(range(NC))]

    with tile.TileContext(nc) as tc, ExitStack() as top:
        dram = top.enter_context(tc.tile_pool(name="dram", bufs=1, space="DRAM"))
        # DRAM scratch
        hext_loc = dram.tile([SLICE + 1, FA], BF16)
        h2_loc = dram.tile([SLICE + 1, FA2], BF16)
        adc1_t = dram.tile([SLICE + 1, H], BF16)
        adc2_t = dram.tile([SLICE + 1, 1], BF16)
        hext_tab = dram.tile([NTAB, FA], BF16)
        h2_tab = dram.tile([NTAB, FA2], BF16)
        hext_rep = dram.tile([NC, SLICE + 1, FA], BF16)
        h2_rep = dram.tile([NC, SLICE + 1, FA2], BF16)

        cpool = top.enter_context(tc.tile_pool(name="consts", bufs=1))
        w1t = cpool.tile([128, KT, F], BF16)
        nc.sync.dma_start(out=w1t[:], in_=w1t_d[:])
        ablk = cpool.tile([F, 2 * H], BF16)
        nc.sync.dma_start(out=ablk[:], in_=ablk_d[:])
        w2e = cpool.tile([F, C + 2], BF16)
        nc.sync.dma_start(out=w2e[:], in_=w2e_d[:])
        rconst = cpool.tile([128, 128], F32)
        nc.sync.dma_start(out=rconst[:], in_=rconst_d[:])
        ident = cpool.tile([128, 128], BF16)
        nc.sync.dma_start(out=ident[:], in_=ident_d[:])
        b1rep = cpool.tile([128, F], F32)
        nc.sync.dma_start(out=b1rep[:], in_=b1rep_d[:])
        b2rep = cpool.tile([128, C], F32)
        nc.sync.dma_start(out=b2rep[:], in_=b2rep_d[:])
        # persistent edge-index tiles
        src_tab = cpool.tile([128, NBB], I32)
        nc.sync.dma_start(out=src_tab[:], in_=src_tab_d[:])
        adc_idx = cpool.tile([128, NBB], I32)
        nc.sync.dma_start(out=adc_idx[:], in_=adc_idx_d[:])
        dsl = cpool.tile([128, NBB], F32)
        nc.sync.dma_start(out=dsl[:], in_=dsl_d[:])

        # ---------------- phase 0: node table build ----------------
        with ExitStack() as ph0:
            sb = ph0.enter_context(tc.tile_pool(name="p0sb", bufs=2))
            ps = ph0.enter_context(tc.tile_pool(name="p0ps", bufs=2, space="PSUM"))
            NJ = TILE_N // 128
            for t in range(T0):
                xt = sb.tile([128, KT, TILE_N], BF16, tag="xt")
                nc.sync.dma_start(
                    out=xt[:],
                    in_=xt_d[:, t * TILE_N:(t + 1) * TILE_N]
                    .rearrange("(i p) n -> p i n", p=128))
                psum_h = ps.tile([128, TILE_N], F32, tag="ph")
                for i in range(KT):
                    nc.tensor.matmul(psum_h[:], lhsT=w1t[:, i, :], rhs=xt[:, i, :],
                                     start=(i == 0), stop=(i == KT - 1))
                hsb = sb.tile([F, TILE_N], BF16, tag="hsb")
                nc.vector.tensor_copy(out=hsb[:], in_=psum_h[:F, :])
                psum_aa = ps.tile([2 * H, TILE_N], F32, tag="paa")
                nc.tensor.matmul(psum_aa[:], lhsT=ablk[:], rhs=hsb[:],
                                 start=True, stop=True)
                aasb = sb.tile([2 * H, TILE_N], BF16, tag="aasb")
                nc.vector.tensor_copy(out=aasb[:], in_=psum_aa[:])
                psum_hT = ps.tile([128, NJ, 128], BF16, tag="pht")
                for j in range(NJ):
                    nc.tensor.transpose(out=psum_hT[:, j, :F],
                                        in_=hsb[:, j * 128:(j + 1) * 128],
                                        identity=ident[:])
                psum_aaT = ps.tile([128, NJ, 2 * H], BF16, tag="paat")
                for j in range(NJ):
                    nc.tensor.transpose(out=psum_aaT[:, j, :],
                                        in_=aasb[:, j * 128:(j + 1) * 128],
                                        identity=ident[:2 * H, :2 * H])
                hx = sb.tile([128, NJ, FA], BF16, tag="hx")
                nc.vector.tensor_copy(out=hx[:, :, 0:F],
                                      in_=psum_hT[:, :, :F])
                nc.vector.tensor_copy(out=hx[:, :, F:FA],
                                      in_=psum_aaT[:, :, 0:H])
                nc.sync.dma_start(
                    out=hext_loc[t * TILE_N:(t + 1) * TILE_N, :]
                    .rearrange("(j p) c -> p j c", p=128),
                    in_=hx[:])
                adcsb = sb.tile([128, NJ, H], BF16, tag="adcsb")
                nc.vector.tensor_copy(out=adcsb[:], in_=psum_aaT[:, :, H:2 * H])
                nc.sync.dma_start(
                    out=adc1_t[t * TILE_N:(t + 1) * TILE_N, :]
                    .rearrange("(j p) c -> p j c", p=128),
                    in_=adcsb[:])

        # sentinel rows (adc sentinels are zero rows), then all-gather
        nc.sync.dma_start(out=hext_loc[SLICE:SLICE + 1, :], in_=sent1_d[:])
        nc.sync.dma_start(out=adc1_t[SLICE:SLICE + 1, :], in_=sent1_d[:, 0:H])
        nc.sync.dma_start(out=adc2_t[SLICE:SLICE + 1, :], in_=sent1_d[:, 0:1])
        # zero adc2_t tail rows (rows >= NPC are never written by the L1
        # drain); the S.T selection matmul would otherwise contract
        # 0 * uninitialized(NaN) = NaN.  Same write pattern as phase 0.
        zt = cpool.tile([128, 4, 1], BF16)
        nc.vector.memset(zt[:], 0.0)
        nc.sync.dma_start(
            out=adc2_t[SLICE - 512:SLICE, :].rearrange("(j p) c -> p j c", p=128),
            in_=zt[:])
        ag1_delay_last = None
        if NC == 1:
            nc.sync.dma_start(out=hext_tab[:], in_=hext_loc[:])
        else:
            # emulate AllGather with AllToAll (AllGather is broken in this
            # runtime): every rank sends its slice to all ranks
            for j in range(NC):
                nc.sync.dma_start(out=hext_rep[j], in_=hext_loc[:])
            nc.gpsimd.collective_compute(
                "AllToAll", OP.bypass, replica_groups=rg,
                ins=[hext_rep[:].rearrange("c r f -> c (r f)").opt()],
                outs=[hext_tab[:].rearrange("(c r) f -> c (r f)", c=NC).opt()])

        # ---------------- layer-1 edge phase ----------------
        with ExitStack() as ph1:
            sb = ph1.enter_context(tc.tile_pool(name="l1sb", bufs=2))
            gpool = ph1.enter_context(tc.tile_pool(name="l1gb", bufs=3))
            ps = ph1.enter_context(tc.tile_pool(name="l1ps", bufs=2, space="PSUM"))
            ps2 = ph1.enter_context(tc.tile_pool(name="l1ps2", bufs=1, space="PSUM"))
            ps3 = ph1.enter_context(tc.tile_pool(name="l1ps3", bufs=2, space="PSUM"))
            ps4 = ph1.enter_context(tc.tile_pool(name="l1ps4", bufs=1, space="PSUM"))
            drsb = ph1.enter_context(tc.tile_pool(name="l1dr", bufs=2))
            for ch in range(NCH):
                db0 = ch * CHUNK_DB
                CB = min(CHUNK_DB, NB - db0)
                c0, c1 = db0 * B, (db0 + CB) * B
                JW = CB * B
                gbuf = gpool.tile([128, CHUNK_DB * B, FA], BF16, tag="gbuf")
                for j in range(JW):
                    nc.gpsimd.indirect_dma_start(
                        out=gbuf[:, j, :], out_offset=None,
                        in_=hext_tab[:],
                        in_offset=bass.IndirectOffsetOnAxis(
                            ap=src_tab[:, c0 + j:c0 + j + 1], axis=0))
                # S one-hot
                S = sb.tile([128, CHUNK_DB * B, 128], BF16, tag="S")
                nc.vector.tensor_tensor(
                    out=S[:, 0:JW, :],
                    in0=dsl[:, c0:c1].unsqueeze(2).to_broadcast([128, JW, 128]),
                    in1=rconst[:].unsqueeze(1).to_broadcast([128, JW, 128]),
                    op=OP.is_equal)
                # per-edge adc selected on TensorE instead of gathered:
                # adcE[:, j, :] = S_j.T @ adcblk  (S_j.T via transpose)
                adcb4 = sb.tile([128, CHUNK_DB, H], BF16, tag="adcb4")
                nc.sync.dma_start(
                    out=adcb4[:, 0:CB, :],
                    in_=adc1_t[db0 * 128:(db0 + CB) * 128, :]
                    .rearrange("(b p) h -> p b h", p=128))
                adcE = sb.tile([128, CHUNK_DB * B, H], BF16, tag="adcE")
                for q in range(0, JW, 4):
                    qw = min(4, JW - q)
                    pst = ps3.tile([128, 4, 128], BF16, tag="pst")
                    for k in range(qw):
                        nc.tensor.transpose(out=pst[:, k, :], in_=S[:, q + k, :],
                                            identity=ident[:])
                    stb = sb.tile([128, 4, 128], BF16, tag="stb")
                    nc.vector.tensor_copy(out=stb[:, 0:qw, :], in_=pst[:, 0:qw, :])
                    for k0 in range(0, qw, 2):
                        kw = min(2, qw - k0)
                        # [128, 2, 512] f32: each k-slice is one full PSUM bank,
                        # keeping matmul outputs bank-aligned
                        pad = ps4.tile([128, 2, 512], F32, tag="pad")
                        for k in range(k0, k0 + kw):
                            nc.tensor.matmul(pad[:, k - k0, 0:H],
                                             lhsT=stb[:, k, :],
                                             rhs=adcb4[:, (q + k) // B, :],
                                             start=True, stop=True)
                        nc.vector.tensor_copy(out=adcE[:, q + k0:q + k0 + kw, :],
                                              in_=pad[:, 0:kw, 0:H])
                # p = exp(lrelu(asc+adc)) ; bf16
                ee = sb.tile([128, CHUNK_DB * B, H], F32, tag="ee")
                nc.vector.tensor_tensor(out=ee[:, 0:JW, :],
                                        in0=gbuf[:, 0:JW, F:FA],
                                        in1=adcE[:, 0:JW, :], op=OP.add)
                nc.vector.scalar_tensor_tensor(
                    out=ee[:, 0:JW, :], in0=ee[:, 0:JW, :], scalar=0.2,
                    in1=ee[:, 0:JW, :], op0=OP.mult, op1=OP.max)
                pt = sb.tile([128, CHUNK_DB * B, H], BF16, tag="pt")
                nc.scalar.activation(pt[:, 0:JW, :], ee[:, 0:JW, :], AF.Exp)
                # rhs = [h*p | p]
                rhs = sb.tile([128, CHUNK_DB * B, FR], BF16, tag="rhs")
                nc.vector.tensor_tensor(
                    out=rhs[:, 0:JW, 0:F].rearrange("p j (h c) -> p j h c", c=cfg.HID),
                    in0=gbuf[:, 0:JW, 0:F].rearrange("p j (h c) -> p j h c", c=cfg.HID),
                    in1=pt[:, 0:JW, :].unsqueeze(3).to_broadcast([128, JW, H, cfg.HID]),
                    op=OP.mult)
                nc.vector.tensor_copy(out=rhs[:, 0:JW, F:FR], in_=pt[:, 0:JW, :])
                if debug_taps and ch == 0:
                    nc.sync.dma_start(out=dbg_gbuf[:, 0:JW, :], in_=gbuf[:, 0:JW, :])
                    nc.sync.dma_start(out=dbg_adcb[:, 0:JW, :], in_=adcE[:, 0:JW, :])
                    nc.sync.dma_start(out=dbg_pt[:, 0:JW, :], in_=pt[:, 0:JW, :])
                    nc.sync.dma_start(out=dbg_S[:, 0:JW, :], in_=S[:, 0:JW, :])

                for lb in range(CB):
                    db = db0 + lb
                    acc = ps.tile([128, FR], F32, tag="acc")
                    for j in range(B):
                        jj = lb * B + j
                        nc.tensor.matmul(acc[:], lhsT=S[:, jj, :],
                                         rhs=rhs[:, jj, :],
                                         start=(j == 0), stop=(j == B - 1))
                    # drain: out1 = acc[:, :F]/denom + b1 ; relu
                    rec = drsb.tile([128, H], F32, tag="rec")
                    nc.vector.tensor_scalar_add(rec[:], acc[:, F:FR], 1e-16)
                    nc.vector.reciprocal(rec[:], rec[:])
                    o1 = drsb.tile([128, F], F32, tag="o1")
                    nc.vector.tensor_tensor(
                        out=o1[:].rearrange("p (h c) -> p h c", c=cfg.HID),
                        in0=acc[:, 0:F].rearrange("p (h c) -> p h c", c=cfg.HID),
                        in1=rec[:].unsqueeze(2).to_broadcast([128, H, cfg.HID]),
                        op=OP.mult)
                    nc.vector.tensor_add(out=o1[:], in0=o1[:], in1=b1rep[:])
                    r1 = drsb.tile([128, F], BF16, tag="r1")
                    nc.scalar.activation(r1[:], o1[:], AF.Relu)
                    # h2 rows: transpose r1, matmul with W2ext
                    pt1 = ps2.tile([128, 128], BF16, tag="pt1")
                    nc.tensor.transpose(out=pt1[:, :F], in_=r1[:], identity=ident[:])
                    r1T = drsb.tile([F, 128], BF16, tag="r1T")
                    nc.vector.tensor_copy(out=r1T[:], in_=pt1[:F, :])
                    ph2 = ps2.tile([128, C + 2], F32, tag="ph2")
                    nc.tensor.matmul(ph2[:], lhsT=r1T[:], rhs=w2e[:],
                                     start=True, stop=True)
                    h2x = drsb.tile([128, FA2], BF16, tag="h2x")
                    nc.vector.tensor_copy(out=h2x[:], in_=ph2[:, 0:FA2])
                    nv = 128 if db < NB - 1 else LASTV
                    nc.sync.dma_start(
                        out=h2_loc[db * 128:db * 128 + nv, :], in_=h2x[:nv, :])
                    a2x = drsb.tile([128, 1], BF16, tag="a2x")
                    nc.vector.tensor_copy(out=a2x[:], in_=ph2[:, FA2:C + 2])
                    nc.sync.dma_start(
                        out=adc2_t[db * 128:db * 128 + nv, :], in_=a2x[:nv, :])

        # sentinel row, then all-gather layer-2 node table
        nc.sync.dma_start(out=h2_loc[SLICE:SLICE + 1, :], in_=sent2_d[:])
        ag2_delay_last = None
        if NC == 1:
            nc.sync.dma_start(out=h2_tab[:], in_=h2_loc[:])
        else:
            for j in range(NC):
                nc.sync.dma_start(out=h2_rep[j], in_=h2_loc[:])
            nc.gpsimd.collective_compute(
                "AllToAll", OP.bypass, replica_groups=rg,
                ins=[h2_rep[:].rearrange("c r f -> c (r f)").opt()],
                outs=[h2_tab[:].rearrange("(c r) f -> c (r f)", c=NC).opt()])

        if debug_taps:
            nc.sync.dma_start(out=dbg_hext[:], in_=hext_tab[:])
            nc.sync.dma_start(out=dbg_adc1[:], in_=adc1_t[:])
            nc.sync.dma_start(out=dbg_h2[:], in_=h2_loc[:])

        # ---------------- layer-2 edge phase ----------------
        with ExitStack() as ph2s:
            sb = ph2s.enter_context(tc.tile_pool(name="l2sb", bufs=2))
            gpool = ph2s.enter_context(tc.tile_pool(name="l2gb", bufs=3))
            ps = ph2s.enter_context(tc.tile_pool(name="l2ps", bufs=2, space="PSUM"))
            ps3 = ph2s.enter_context(tc.tile_pool(name="l2ps3", bufs=2, space="PSUM"))
            ps4 = ph2s.enter_context(tc.tile_pool(name="l2ps4", bufs=1, space="PSUM"))
            drsb = ph2s.enter_context(tc.tile_pool(name="l2dr", bufs=2))
            for ch in range(NCH):
                db0 = ch * CHUNK_DB
                CB = min(CHUNK_DB, NB - db0)
                c0, c1 = db0 * B, (db0 + CB) * B
                JW = CB * B
                g2 = gpool.tile([128, CHUNK_DB * B, FA2], BF16, tag="g2")
                for j in range(JW):
                    nc.gpsimd.indirect_dma_start(
                        out=g2[:, j, :], out_offset=None,
                        in_=h2_tab[:],
                        in_offset=bass.IndirectOffsetOnAxis(
                            ap=src_tab[:, c0 + j:c0 + j + 1], axis=0))
                S = sb.tile([128, CHUNK_DB * B, 128], BF16, tag="S2")
                nc.vector.tensor_tensor(
                    out=S[:, 0:JW, :],
                    in0=dsl[:, c0:c1].unsqueeze(2).to_broadcast([128, JW, 128]),
                    in1=rconst[:].unsqueeze(1).to_broadcast([128, JW, 128]),
                    op=OP.is_equal)
                a2b4 = sb.tile([128, CHUNK_DB, 1], BF16, tag="a2b4")
                nc.sync.dma_start(
                    out=a2b4[:, 0:CB, :],
                    in_=adc2_t[db0 * 128:(db0 + CB) * 128, :]
                    .rearrange("(b p) h -> p b h", p=128))
                a2E = sb.tile([128, CHUNK_DB * B, 1], BF16, tag="a2E")
                for q in range(0, JW, 4):
                    qw = min(4, JW - q)
                    pst = ps3.tile([128, 4, 128], BF16, tag="pst2")
                    for k in range(qw):
                        nc.tensor.transpose(out=pst[:, k, :], in_=S[:, q + k, :],
                                            identity=ident[:])
                    stb = sb.tile([128, 4, 128], BF16, tag="stb2")
                    nc.vector.tensor_copy(out=stb[:, 0:qw, :], in_=pst[:, 0:qw, :])
                    for k0 in range(0, qw, 2):
                        kw = min(2, qw - k0)
                        pad = ps4.tile([128, 2, 512], F32, tag="pad2")
                        for k in range(k0, k0 + kw):
                            nc.tensor.matmul(pad[:, k - k0, 0:1],
                                             lhsT=stb[:, k, :],
                                             rhs=a2b4[:, (q + k) // B, :],
                                             start=True, stop=True)
                        nc.vector.tensor_copy(out=a2E[:, q + k0:q + k0 + kw, :],
                                              in_=pad[:, 0:kw, 0:1])
                ee = sb.tile([128, CHUNK_DB * B, 1], F32, tag="ee2")
                nc.vector.tensor_tensor(out=ee[:, 0:JW, :],
                                        in0=g2[:, 0:JW, C:FA2],
                                        in1=a2E[:, 0:JW, :], op=OP.add)
                nc.vector.scalar_tensor_tensor(
                    out=ee[:, 0:JW, :], in0=ee[:, 0:JW, :], scalar=0.2,
                    in1=ee[:, 0:JW, :], op0=OP.mult, op1=OP.max)
                pt = sb.tile([128, CHUNK_DB * B, 1], BF16, tag="pt2")
                nc.scalar.activation(pt[:, 0:JW, :], ee[:, 0:JW, :], AF.Exp)
                rhs = sb.tile([128, CHUNK_DB * B, FR2], BF16, tag="rhs2")
                nc.vector.tensor_tensor(
                    out=rhs[:, 0:JW, 0:C],
                    in0=g2[:, 0:JW, 0:C],
                    in1=pt[:, 0:JW, :].to_broadcast([128, JW, C]),
                    op=OP.mult)
                nc.vector.tensor_copy(out=rhs[:, 0:JW, C:FR2], in_=pt[:, 0:JW, :])

                for lb in range(CB):
                    db = db0 + lb
                    acc = ps.tile([128, FR2], F32, tag="acc2")
                    for j in range(B):
                        jj = lb * B + j
                        nc.tensor.matmul(acc[:], lhsT=S[:, jj, :],
                                         rhs=rhs[:, jj, :],
                                         start=(j == 0), stop=(j == B - 1))
                    rec = drsb.tile([128, 1], F32, tag="rec2")
                    nc.vector.tensor_scalar_add(rec[:], acc[:, C:FR2], 1e-16)
                    nc.vector.reciprocal(rec[:], rec[:])
                    o2 = drsb.tile([128, C], F32, tag="o2")
                    nc.vector.tensor_tensor(
                        out=o2[:], in0=acc[:, 0:C],
                        in1=rec[:].to_broadcast([128, C]), op=OP.mult)
                    nc.vector.tensor_add(out=o2[:], in0=o2[:], in1=b2rep[:])
                    # log_softmax
                    mneg = drsb.tile([128, 1], F32, tag="mneg")
                    nc.vector.tensor_reduce(out=mneg[:], in_=o2[:], axis=AX.X,
                                            op=OP.max, negate=True)
                    escr = drsb.tile([128, C], F32, tag="escr")
                    ssum = drsb.tile([128, 1], F32, tag="ssum")
                    nc.scalar.activation(escr[:], o2[:], AF.Exp,
                                         bias=mneg[:, 0:1], accum_out=ssum[:])
                    lns = drsb.tile([128, 1], F32, tag="lns")
                    nc.scalar.activation(lns[:], ssum[:], AF.Ln)
                    tsh = drsb.tile([128, 1], F32, tag="tsh")
                    nc.vector.tensor_sub(out=tsh[:], in0=mneg[:], in1=lns[:])
                    fin = drsb.tile([128, C], F32, tag="fin")
                    nc.vector.tensor_scalar(out=fin[:], in0=o2[:],
                                            scalar1=tsh[:, 0:1], scalar2=None,
                                            op0=OP.add)
                    nv = 128 if db < NB - 1 else LASTV
                    nc.sync.dma_start(out=out_d[db * 128:db * 128 + nv, :],
                                      in_=fin[:nv, :])

    nc.compile()
    return nc


_NC_CACHE: dict = {}


def _get_nc(cfg: Cfg):
    if cfg not in _NC_CACHE:
        _NC_CACHE[cfg] = build_nc(cfg)
    return _NC_CACHE[cfg]


def kernel(x, edge_index, W1, a_src1, a_dst1, b1, W2, a_src2, a_dst2, b2,
           cfg: Cfg | None = None, _run=None):
    x = np.asarray(x)
    edge_index = np.asarray(edge_index)
    if cfg is None:
        cfg = Cfg()
        loops = np.arange(cfg.N, dtype=np.int64)
        dst = np.concatenate([np.asarray(edge_index[1]).astype(np.int64), loops])
        b = compute_B(dst, cfg)
        if b != cfg.B:
            cfg = Cfg(B=b)
    in_maps = build_in_maps(x, edge_index, W1, a_src1, a_dst1, b1,
                            W2, a_src2, a_dst2, b2, cfg)
    nc = _get_nc(cfg)
    if _run is not None:
        results = _run(nc, in_maps)
    else:
        res = run_bass_kernel_spmd(nc, in_maps, list(range(cfg.NC)))
        results = res.results
    out = np.concatenate([results[c]["out"] for c in range(cfg.NC)], axis=0)
    return out.astype(np.float32)



# revision 35
# speedup vs baseline: 1.0066x; 1.0066x over previous
"""GAT 2-layer network (PyG GATConv x2 + log_softmax) on 8 Trainium2 NeuronCores.

Strategy (dst-sharded message passing):
  - Nodes are 1D-sharded across the 8 cores (6250 nodes/core). Edges (with
    self-loops) are assigned to the core owning their *destination* node.
  - Phase 0 (per core): h = x @ W1.T for the core's node slice, plus the
    per-node attention scalars asc = (h . a_src), adc = (h . a_dst), computed
    with TensorE matmuls.  Rows [h | asc] are written to a DRAM table in bf16.
  - AllGather the node table so every core has all nodes' rows.
  - Edge phase (per core): edges sorted by dst, grouped into 128-dst-node
    "dst-blocks", each padded to a uniform number B of 128-edge blocks.
    For each chunk of dst-blocks:
      * indirect-DMA gather of [h|asc] rows by src index (edge-major tiles)
      * indirect-DMA gather of adc rows by (local) dst index
      * p = exp(leaky_relu(asc_src + adc_dst))  (softmax max-subtraction is
        skipped; mathematically identical, values are small)
      * one-hot selection matrix S[e, d] = (dstslot[e] == d) built on DVE
      * PSUM-accumulated matmuls  S.T @ [h*p | p]  give, per dst node, both
        sum_e p_e * h_src_e  and the softmax denominator sum_e p_e.
      * drain: out = (sum p h)/denom + b, relu -> layer-2 rows h2 = r1 @ W2.T
        (+ fused asc2/adc2 via extra columns of W2) written to a second table.
  - AllGather the layer-2 table, run the same edge phase with 32 features and
    1 head, finish with log_softmax per node, write the output slice.

The kernel() entry point takes the FULL inputs and returns the FULL output.
"""

import math
from contextlib import ExitStack
from dataclasses import dataclass

import numpy as np

import concourse.bass as bass
import concourse.mybir as mybir
import concourse.tile as tile
from concourse import bacc
from concourse.tile_rust import add_dep_helper
from concourse.bass_utils import run_bass_kernel_spmd

F32 = mybir.dt.float32
BF16 = mybir.dt.bfloat16
I32 = mybir.dt.int32
AX = mybir.AxisListType
OP = mybir.AluOpType
AF = mybir.ActivationFunctionType

SENT_ASC = -200.0  # sentinel asc value: exp(lrelu(-200+adc)) ~= e^-40 ~ 4e-18


@dataclass(frozen=True)
class Cfg:
    N: int = 50000
    F_IN: int = 256
    HID: int = 32
    HEADS: int = 4
    CLASSES: int = 32
    NC: int = 8
    TILE_N: int = 512   # phase-0 node tile
    CHUNK_DB: int = 5   # dst-blocks per gather chunk
    B: int = 18         # uniform 128-edge blocks per dst-block (data dependent)
    GB: int = 1         # gather columns per indirect DMA (HW supports only 1)

    @property
    def F(self):  # layer-1 feature width
        return self.HID * self.HEADS

    @property
    def NPC(self):
        return self.N // self.NC

    @property
    def T0(self):
        return math.ceil(self.NPC / self.TILE_N)

    @property
    def SLICE(self):
        return self.T0 * self.TILE_N

    @property
    def NB(self):  # dst-blocks per core
        return math.ceil(self.NPC / 128)

    @property
    def SLICEP(self):  # per-core table slice incl per-core sentinel row
        return self.SLICE + 1

    @property
    def NTAB(self):  # node-table rows (NC slices, each with sentinel)
        return self.NC * self.SLICEP

    @property
    def NCH(self):  # gather chunks per core
        return math.ceil(self.NB / self.CHUNK_DB)

    @property
    def FA(self):  # layer-1 table row width: h(F) + asc(HEADS)
        return self.F + self.HEADS

    @property
    def FR(self):  # layer-1 rhs width: h*p(F) + p(HEADS)
        return self.F + self.HEADS

    @property
    def FA2(self):  # layer-2 table row width: h2(CLASSES) + asc2(1)
        return self.CLASSES + 1

    @property
    def FR2(self):  # layer-2 rhs width
        return self.CLASSES + 1


def compute_B(dst, cfg: Cfg):
    """Max 128-edge blocks needed by any (core, dst-block)."""
    NPC, NB = cfg.NPC, cfg.NB
    core = dst // NPC
    blk = (dst % NPC) // 128
    cnt = np.bincount(core * NB + blk, minlength=cfg.NC * NB)
    return int(np.max((cnt + 127) // 128))


def build_layout(src, dst, cfg: Cfg):
    """Per-core index arrays, laid out as [128, NB*B] (partition-major)."""
    NC, NPC, NB, B = cfg.NC, cfg.NPC, cfg.NB, cfg.B
    SLICEP = cfg.SLICEP
    NBB = NB * B
    SENT_ROW = cfg.SLICE  # core 0's sentinel row

    core = dst // NPC
    local = dst % NPC
    blk = local // 128
    seg = core * NB + blk
    order = np.argsort(seg, kind="stable")
    seg_s = seg[order]
    src_s = src[order]
    local_s = local[order]

    cnt = np.bincount(seg, minlength=NC * NB)
    seg_start = np.zeros(NC * NB + 1, dtype=np.int64)
    np.cumsum(cnt, out=seg_start[1:])
    pos = np.arange(len(dst)) - seg_start[seg_s]
    assert pos.max() < B * 128, "B too small for this edge distribution"

    # target coordinates inside the per-core [128, NB*B] arrays
    c_s = seg_s // NB
    blk_s = seg_s % NB
    p_s = pos % 128
    j_s = blk_s * B + pos // 128

    src_tab = np.full((NC, 128, NBB), SENT_ROW, dtype=np.int32)
    adc_idx = np.full((NC, 128, NBB), cfg.SLICE, dtype=np.int32)  # zero sentinel row
    dsl = np.zeros((NC, 128, NBB), dtype=np.float32)

    src_core = src_s // NPC
    src_local = src_s % NPC
    src_coord = (src_core * SLICEP + src_local).astype(np.int32)

    src_tab[c_s, p_s, j_s] = src_coord
    adc_idx[c_s, p_s, j_s] = local_s.astype(np.int32)
    dsl[c_s, p_s, j_s] = (local_s - blk_s * 128).astype(np.float32)
    return src_tab, adc_idx, dsl


def pack_consts(W1, a_src1, a_dst1, b1, W2, a_src2, a_dst2, b2, cfg: Cfg):
    F, H, HID, C, F_IN = cfg.F, cfg.HEADS, cfg.HID, cfg.CLASSES, cfg.F_IN
    KT = F_IN // 128
    consts = {}
    # W1T[p, i, f] = W1[f, i*128+p]
    w1t = W1.T.reshape(KT, 128, F).transpose(1, 0, 2)
    consts["w1t"] = w1t.astype(np.float32)  # cast to bf16 happens at DMA? no: stored bf16 below
    # a blockdiag: [F, 2H]: col j<H -> a_src1[j], col j>=H -> a_dst1[j-H]
    ablk = np.zeros((F, 2 * H), dtype=np.float32)
    for h in range(H):
        ablk[h * HID:(h + 1) * HID, h] = a_src1[h]
        ablk[h * HID:(h + 1) * HID, H + h] = a_dst1[h]
    consts["ablk"] = ablk
    # W2ext [F, C+2]: W2.T | (a_src2@W2).T | (a_dst2@W2).T
    w2e = np.concatenate(
        [W2.T.astype(np.float64),
         (a_src2 @ W2).T.astype(np.float64),
         (a_dst2 @ W2).T.astype(np.float64)], axis=1)
    consts["w2e"] = w2e.astype(np.float32)
    consts["rconst"] = np.tile(np.arange(128, dtype=np.float32)[None, :], (128, 1))
    consts["ident"] = np.eye(128, dtype=np.float32)
    consts["b1rep"] = np.tile(b1[None, :].astype(np.float32), (128, 1))
    consts["b2rep"] = np.tile(b2[None, :].astype(np.float32), (128, 1))
    sent1 = np.zeros((1, cfg.FA), dtype=np.float32)
    sent1[0, F:] = SENT_ASC
    consts["sent1"] = sent1
    sent2 = np.zeros((1, cfg.FA2), dtype=np.float32)
    sent2[0, C:] = SENT_ASC
    consts["sent2"] = sent2
    return consts


def build_in_maps(x, edge_index, W1, a_src1, a_dst1, b1, W2, a_src2, a_dst2, b2,
                  cfg: Cfg):
    N, NC, NPC, SLICE = cfg.N, cfg.NC, cfg.NPC, cfg.SLICE
    loops = np.arange(N, dtype=edge_index.dtype)
    src = np.concatenate([np.asarray(edge_index[0]), loops]).astype(np.int64)
    dst = np.concatenate([np.asarray(edge_index[1]), loops]).astype(np.int64)

    src_tab, adc_idx, dsl = build_layout(src, dst, cfg)
    consts = pack_consts(np.asarray(W1), np.asarray(a_src1), np.asarray(a_dst1),
                         np.asarray(b1), np.asarray(W2), np.asarray(a_src2),
                         np.asarray(a_dst2), np.asarray(b2), cfg)

    xT = np.zeros((cfg.F_IN, NC * SLICE), dtype=np.float32)
    xv = np.asarray(x).T  # [F_IN, N]
    for c in range(NC):
        xT[:, c * SLICE:c * SLICE + NPC] = xv[:, c * NPC:(c + 1) * NPC]

    bf = np.dtype("bfloat16") if False else None  # bf16 handled via ml_dtypes below
    import ml_dtypes
    to_bf16 = lambda a: a.astype(ml_dtypes.bfloat16)

    in_maps = []
    for c in range(NC):
        m = {
            "xt": to_bf16(xT[:, c * SLICE:(c + 1) * SLICE]),
            "src_tab": src_tab[c],
            "adc_idx": adc_idx[c],
            "dsl": dsl[c],
            "w1t": to_bf16(consts["w1t"]),
            "ablk": to_bf16(consts["ablk"]),
            "w2e": to_bf16(consts["w2e"]),
            "rconst": consts["rconst"],
            "ident": to_bf16(consts["ident"]),
            "b1rep": consts["b1rep"],
            "b2rep": consts["b2rep"],
            "sent1": to_bf16(consts["sent1"]),
            "sent2": to_bf16(consts["sent2"]),
        }
        in_maps.append(m)
    return in_maps


def _delay_chain(nc, tc, ctx, n):
    if not n:
        return None
    pool = ctx.enter_context(tc.tile_pool(name=f"dly{nc.next_id()}", bufs=1))
    t = pool.tile([128, 8192], F32, name=f"dlyt{nc.next_id()}")
    last = nc.vector.memset(t[:], 0.0)
    for _ in range(n):
        last = nc.vector.tensor_scalar_add(t[:], t[:], 1.0)
    return last


def build_nc(cfg: Cfg, debug_taps: bool = False, ag_delay: int = 0):
    """Build + compile the SPMD program (identical for all cores)."""
    N, NC, NPC, NB, B, SLICE, T0 = (cfg.N, cfg.NC, cfg.NPC, cfg.NB, cfg.B,
                                    cfg.SLICE, cfg.T0)
    F, H, C, F_IN = cfg.F, cfg.HEADS, cfg.CLASSES, cfg.F_IN
    FA, FR, FA2, FR2 = cfg.FA, cfg.FR, cfg.FA2, cfg.FR2
    TILE_N, CHUNK_DB, NCH = cfg.TILE_N, cfg.CHUNK_DB, cfg.NCH
    NBB = NB * B
    KT = F_IN // 128
    NTAB = cfg.NTAB
    LASTV = NPC - (NB - 1) * 128  # valid rows in last dst-block

    nc = bacc.Bacc("TRN2", target_bir_lowering=False, debug=False,
                   num_devices=NC)

    dt = nc.dram_tensor
    xt_d = dt("xt", [F_IN, SLICE], BF16, kind="ExternalInput").ap()
    src_tab_d = dt("src_tab", [128, NBB], I32, kind="ExternalInput").ap()
    adc_idx_d = dt("adc_idx", [128, NBB], I32, kind="ExternalInput").ap()
    dsl_d = dt("dsl", [128, NBB], F32, kind="ExternalInput").ap()
    w1t_d = dt("w1t", [128, KT, F], BF16, kind="ExternalInput").ap()
    ablk_d = dt("ablk", [F, 2 * H], BF16, kind="ExternalInput").ap()
    w2e_d = dt("w2e", [F, C + 2], BF16, kind="ExternalInput").ap()
    rconst_d = dt("rconst", [128, 128], F32, kind="ExternalInput").ap()
    ident_d = dt("ident", [128, 128], BF16, kind="ExternalInput").ap()
    b1rep_d = dt("b1rep", [128, F], F32, kind="ExternalInput").ap()
    b2rep_d = dt("b2rep", [128, C], F32, kind="ExternalInput").ap()
    sent1_d = dt("sent1", [1, FA], BF16, kind="ExternalInput").ap()
    sent2_d = dt("sent2", [1, FA2], BF16, kind="ExternalInput").ap()
    out_d = dt("out", [NPC, C], F32, kind="ExternalOutput").ap()
    if debug_taps:
        dbg_hext = dt("dbg_hext", [NTAB, FA], BF16, kind="ExternalOutput").ap()
        dbg_adc1 = dt("dbg_adc1", [SLICE + 1, H], BF16, kind="ExternalOutput").ap()
        dbg_h2 = dt("dbg_h2", [SLICE + 1, FA2], BF16, kind="ExternalOutput").ap()
        dbg_gbuf = dt("dbg_gbuf", [128, CHUNK_DB * B, FA], BF16, kind="ExternalOutput").ap()
        dbg_adcb = dt("dbg_adcb", [128, CHUNK_DB * B, H], BF16, kind="ExternalOutput").ap()
        dbg_pt = dt("dbg_pt", [128, CHUNK_DB * B, H], BF16, kind="ExternalOutput").ap()
        dbg_S = dt("dbg_S", [128, CHUNK_DB * B, 128], BF16, kind="ExternalOutput").ap()

    rg = [list(range(NC))]

    with tile.TileContext(nc) as tc, ExitStack() as top:
        dram = top.enter_context(tc.tile_pool(name="dram", bufs=1, space="DRAM"))
        # DRAM scratch
        hext_loc = dram.tile([SLICE + 1, FA], BF16)
        h2_loc = dram.tile([SLICE + 1, FA2], BF16)
        adc1_t = dram.tile([SLICE + 1, H], BF16)
        adc2_t = dram.tile([SLICE + 1, 1], BF16)
        hext_tab = dram.tile([NTAB, FA], BF16)
        h2_tab = dram.tile([NTAB, FA2], BF16)
        hext_rep = dram.tile([NC, SLICE + 1, FA], BF16)
        h2_rep = dram.tile([NC, SLICE + 1, FA2], BF16)

        cpool = top.enter_context(tc.tile_pool(name="consts", bufs=1))
        w1t = cpool.tile([128, KT, F], BF16)
        nc.sync.dma_start(out=w1t[:], in_=w1t_d[:])
        ablk = cpool.tile([F, 2 * H], BF16)
        nc.sync.dma_start(out=ablk[:], in_=ablk_d[:])
        w2e = cpool.tile([F, C + 2], BF16)
        nc.sync.dma_start(out=w2e[:], in_=w2e_d[:])
        rconst = cpool.tile([128, 128], F32)
        nc.sync.dma_start(out=rconst[:], in_=rconst_d[:])
        ident = cpool.tile([128, 128], BF16)
        nc.sync.dma_start(out=ident[:], in_=ident_d[:])
        b1rep = cpool.tile([128, F], F32)
        nc.sync.dma_start(out=b1rep[:], in_=b1rep_d[:])
        b2rep = cpool.tile([128, C], F32)
        nc.sync.dma_start(out=b2rep[:], in_=b2rep_d[:])
        # persistent edge-index tiles
        src_tab = cpool.tile([128, NBB], I32)
        nc.sync.dma_start(out=src_tab[:], in_=src_tab_d[:])
        adc_idx = cpool.tile([128, NBB], I32)
        nc.sync.dma_start(out=adc_idx[:], in_=adc_idx_d[:])
        dsl = cpool.tile([128, NBB], F32)
        nc.sync.dma_start(out=dsl[:], in_=dsl_d[:])

        # ---------------- phase 0: node table build ----------------
        with ExitStack() as ph0:
            sb = ph0.enter_context(tc.tile_pool(name="p0sb", bufs=2))
            ps = ph0.enter_context(tc.tile_pool(name="p0ps", bufs=2, space="PSUM"))
            NJ = TILE_N // 128
            for t in range(T0):
                xt = sb.tile([128, KT, TILE_N], BF16, tag="xt")
                nc.sync.dma_start(
                    out=xt[:],
                    in_=xt_d[:, t * TILE_N:(t + 1) * TILE_N]
                    .rearrange("(i p) n -> p i n", p=128))
                psum_h = ps.tile([128, TILE_N], F32, tag="ph")
                for i in range(KT):
                    nc.tensor.matmul(psum_h[:], lhsT=w1t[:, i, :], rhs=xt[:, i, :],
                                     start=(i == 0), stop=(i == KT - 1))
                hsb = sb.tile([F, TILE_N], BF16, tag="hsb")
                nc.vector.tensor_copy(out=hsb[:], in_=psum_h[:F, :])
                psum_aa = ps.tile([2 * H, TILE_N], F32, tag="paa")
                nc.tensor.matmul(psum_aa[:], lhsT=ablk[:], rhs=hsb[:],
                                 start=True, stop=True)
                aasb = sb.tile([2 * H, TILE_N], BF16, tag="aasb")
                nc.vector.tensor_copy(out=aasb[:], in_=psum_aa[:])
                psum_hT = ps.tile([128, NJ, 128], BF16, tag="pht")
                for j in range(NJ):
                    nc.tensor.transpose(out=psum_hT[:, j, :F],
                                        in_=hsb[:, j * 128:(j + 1) * 128],
                                        identity=ident[:])
                psum_aaT = ps.tile([128, NJ, 2 * H], BF16, tag="paat")
                for j in range(NJ):
                    nc.tensor.transpose(out=psum_aaT[:, j, :],
                                        in_=aasb[:, j * 128:(j + 1) * 128],
                                        identity=ident[:2 * H, :2 * H])
                hx = sb.tile([128, NJ, FA], BF16, tag="hx")
                nc.vector.tensor_copy(out=hx[:, :, 0:F],
                                      in_=psum_hT[:, :, :F])
                nc.vector.tensor_copy(out=hx[:, :, F:FA],
                                      in_=psum_aaT[:, :, 0:H])
                nc.sync.dma_start(
                    out=hext_loc[t * TILE_N:(t + 1) * TILE_N, :]
                    .rearrange("(j p) c -> p j c", p=128),
                    in_=hx[:])
                adcsb = sb.tile([128, NJ, H], BF16, tag="adcsb")
                nc.vector.tensor_copy(out=adcsb[:], in_=psum_aaT[:, :, H:2 * H])
                nc.sync.dma_start(
                    out=adc1_t[t * TILE_N:(t + 1) * TILE_N, :]
                    .rearrange("(j p) c -> p j c", p=128),
                    in_=adcsb[:])

        # sentinel rows (adc sentinels are zero rows), then all-gather
        nc.sync.dma_start(out=hext_loc[SLICE:SLICE + 1, :], in_=sent1_d[:])
        nc.sync.dma_start(out=adc1_t[SLICE:SLICE + 1, :], in_=sent1_d[:, 0:H])
        nc.sync.dma_start(out=adc2_t[SLICE:SLICE + 1, :], in_=sent1_d[:, 0:1])
        # zero adc2_t tail rows (rows >= NPC are never written by the L1
        # drain); the S.T selection matmul would otherwise contract
        # 0 * uninitialized(NaN) = NaN.  Same write pattern as phase 0.
        zt = cpool.tile([128, 4, 1], BF16)
        nc.vector.memset(zt[:], 0.0)
        nc.sync.dma_start(
            out=adc2_t[SLICE - 512:SLICE, :].rearrange("(j p) c -> p j c", p=128),
            in_=zt[:])
        ag1_delay_last = None
        if NC == 1:
            nc.sync.dma_start(out=hext_tab[:], in_=hext_loc[:])
        else:
            # emulate AllGather with AllToAll (AllGather is broken in this
            # runtime): every rank sends its slice to all ranks
            for j in range(NC):
                nc.sync.dma_start(out=hext_rep[j], in_=hext_loc[:])
            nc.gpsimd.collective_compute(
                "AllToAll", OP.bypass, replica_groups=rg,
                ins=[hext_rep[:].rearrange("c r f -> c (r f)").opt()],
                outs=[hext_tab[:].rearrange("(c r) f -> c (r f)", c=NC).opt()])

        # ---------------- layer-1 edge phase ----------------
        with ExitStack() as ph1:
            sb = ph1.enter_context(tc.tile_pool(name="l1sb", bufs=2))
            gpool = ph1.enter_context(tc.tile_pool(name="l1gb", bufs=2))
            ps = ph1.enter_context(tc.tile_pool(name="l1ps", bufs=2, space="PSUM"))
            ps2 = ph1.enter_context(tc.tile_pool(name="l1ps2", bufs=1, space="PSUM"))
            ps3 = ph1.enter_context(tc.tile_pool(name="l1ps3", bufs=2, space="PSUM"))
            drsb = ph1.enter_context(tc.tile_pool(name="l1dr", bufs=2))
            for ch in range(NCH):
                db0 = ch * CHUNK_DB
                CB = min(CHUNK_DB, NB - db0)
                c0, c1 = db0 * B, (db0 + CB) * B
                JW = CB * B
                gbuf = gpool.tile([128, CHUNK_DB * B, FA], BF16, tag="gbuf")
                for j in range(JW):
                    nc.gpsimd.indirect_dma_start(
                        out=gbuf[:, j, :], out_offset=None,
                        in_=hext_tab[:],
                        in_offset=bass.IndirectOffsetOnAxis(
                            ap=src_tab[:, c0 + j:c0 + j + 1], axis=0))
                # S one-hot
                S = sb.tile([128, CHUNK_DB * B, 128], BF16, tag="S")
                nc.vector.tensor_tensor(
                    out=S[:, 0:JW, :],
                    in0=dsl[:, c0:c1].unsqueeze(2).to_broadcast([128, JW, 128]),
                    in1=rconst[:].unsqueeze(1).to_broadcast([128, JW, 128]),
                    op=OP.is_equal)
                # per-edge adc selected on TensorE instead of gathered:
                # adcE[:, j, :] = S_j.T @ adcblk  (S_j.T via transpose)
                adcb4 = sb.tile([128, CHUNK_DB, H], BF16, tag="adcb4")
                nc.sync.dma_start(
                    out=adcb4[:, 0:CB, :],
                    in_=adc1_t[db0 * 128:(db0 + CB) * 128, :]
                    .rearrange("(b p) h -> p b h", p=128))
                adcE = sb.tile([128, CHUNK_DB * B, H], BF16, tag="adcE")
                for q in range(0, JW, 4):
                    qw = min(4, JW - q)
                    pst = ps3.tile([128, 4, 128], BF16, tag="pst")
                    for k in range(qw):
                        nc.tensor.transpose(out=pst[:, k, :], in_=S[:, q + k, :],
                                            identity=ident[:])
                    stb = sb.tile([128, 4, 128], BF16, tag="stb")
                    nc.vector.tensor_copy(out=stb[:, 0:qw, :], in_=pst[:, 0:qw, :])
                    for k in range(qw):
                        pad = ps3.tile([128, H], F32, tag="pad")
                        nc.tensor.matmul(pad[:], lhsT=stb[:, k, :],
                                         rhs=adcb4[:, (q + k) // B, :],
                                         start=True, stop=True)
                        nc.vector.tensor_copy(out=adcE[:, q + k, :], in_=pad[:])
                # p = exp(lrelu(asc+adc)) ; bf16
                ee = sb.tile([128, CHUNK_DB * B, H], F32, tag="ee")
                nc.vector.tensor_tensor(out=ee[:, 0:JW, :],
                                        in0=gbuf[:, 0:JW, F:FA],
                                        in1=adcE[:, 0:JW, :], op=OP.add)
                nc.vector.scalar_tensor_tensor(
                    out=ee[:, 0:JW, :], in0=ee[:, 0:JW, :], scalar=0.2,
                    in1=ee[:, 0:JW, :], op0=OP.mult, op1=OP.max)
                pt = sb.tile([128, CHUNK_DB * B, H], BF16, tag="pt")
                nc.scalar.activation(pt[:, 0:JW, :], ee[:, 0:JW, :], AF.Exp)
                # rhs = [h*p | p]
                rhs = sb.tile([128, CHUNK_DB * B, FR], BF16, tag="rhs")
                nc.vector.tensor_tensor(
                    out=rhs[:, 0:JW, 0:F].rearrange("p j (h c) -> p j h c", c=cfg.HID),
                    in0=gbuf[:, 0:JW, 0:F].rearrange("p j (h c) -> p j h c", c=cfg.HID),
                    in1=pt[:, 0:JW, :].unsqueeze(3).to_broadcast([128, JW, H, cfg.HID]),
                    op=OP.mult)
                nc.vector.tensor_copy(out=rhs[:, 0:JW, F:FR], in_=pt[:, 0:JW, :])
                if debug_taps and ch == 0:
                    nc.sync.dma_start(out=dbg_gbuf[:, 0:JW, :], in_=gbuf[:, 0:JW, :])
                    nc.sync.dma_start(out=dbg_adcb[:, 0:JW, :], in_=adcE[:, 0:JW, :])
                    nc.sync.dma_start(out=dbg_pt[:, 0:JW, :], in_=pt[:, 0:JW, :])
                    nc.sync.dma_start(out=dbg_S[:, 0:JW, :], in_=S[:, 0:JW, :])

                for lb in range(CB):
                    db = db0 + lb
                    acc = ps.tile([128, FR], F32, tag="acc")
                    for j in range(B):
                        jj = lb * B + j
                        nc.tensor.matmul(acc[:], lhsT=S[:, jj, :],
                                         rhs=rhs[:, jj, :],
                                         start=(j == 0), stop=(j == B - 1))
                    # drain: out1 = acc[:, :F]/denom + b1 ; relu
                    rec = drsb.tile([128, H], F32, tag="rec")
                    nc.vector.tensor_scalar_add(rec[:], acc[:, F:FR], 1e-16)
                    nc.vector.reciprocal(rec[:], rec[:])
                    o1 = drsb.tile([128, F], F32, tag="o1")
                    nc.vector.tensor_tensor(
                        out=o1[:].rearrange("p (h c) -> p h c", c=cfg.HID),
                        in0=acc[:, 0:F].rearrange("p (h c) -> p h c", c=cfg.HID),
                        in1=rec[:].unsqueeze(2).to_broadcast([128, H, cfg.HID]),
                        op=OP.mult)
                    nc.vector.tensor_add(out=o1[:], in0=o1[:], in1=b1rep[:])
                    r1 = drsb.tile([128, F], BF16, tag="r1")
                    nc.scalar.activation(r1[:], o1[:], AF.Relu)
                    # h2 rows: transpose r1, matmul with W2ext
                    pt1 = ps2.tile([128, 128], BF16, tag="pt1")
                    nc.tensor.transpose(out=pt1[:, :F], in_=r1[:], identity=ident[:])
                    r1T = drsb.tile([F, 128], BF16, tag="r1T")
                    nc.vector.tensor_copy(out=r1T[:], in_=pt1[:F, :])
                    ph2 = ps2.tile([128, C + 2], F32, tag="ph2")
                    nc.tensor.matmul(ph2[:], lhsT=r1T[:], rhs=w2e[:],
                                     start=True, stop=True)
                    h2x = drsb.tile([128, FA2], BF16, tag="h2x")
                    nc.vector.tensor_copy(out=h2x[:], in_=ph2[:, 0:FA2])
                    nv = 128 if db < NB - 1 else LASTV
                    nc.sync.dma_start(
                        out=h2_loc[db * 128:db * 128 + nv, :], in_=h2x[:nv, :])
                    a2x = drsb.tile([128, 1], BF16, tag="a2x")
                    nc.vector.tensor_copy(out=a2x[:], in_=ph2[:, FA2:C + 2])
                    nc.sync.dma_start(
                        out=adc2_t[db * 128:db * 128 + nv, :], in_=a2x[:nv, :])

        # sentinel row, then all-gather layer-2 node table
        nc.sync.dma_start(out=h2_loc[SLICE:SLICE + 1, :], in_=sent2_d[:])
        ag2_delay_last = None
        if NC == 1:
            nc.sync.dma_start(out=h2_tab[:], in_=h2_loc[:])
        else:
            for j in range(NC):
                nc.sync.dma_start(out=h2_rep[j], in_=h2_loc[:])
            nc.gpsimd.collective_compute(
                "AllToAll", OP.bypass, replica_groups=rg,
                ins=[h2_rep[:].rearrange("c r f -> c (r f)").opt()],
                outs=[h2_tab[:].rearrange("(c r) f -> c (r f)", c=NC).opt()])

        if debug_taps:
            nc.sync.dma_start(out=dbg_hext[:], in_=hext_tab[:])
            nc.sync.dma_start(out=dbg_adc1[:], in_=adc1_t[:])
            nc.sync.dma_start(out=dbg_h2[:], in_=h2_loc[:])

        # ---------------- layer-2 edge phase ----------------
        with ExitStack() as ph2s:
            sb = ph2s.enter_context(tc.tile_pool(name="l2sb", bufs=2))
            gpool = ph2s.enter_context(tc.tile_pool(name="l2gb", bufs=2))
            ps = ph2s.enter_context(tc.tile_pool(name="l2ps", bufs=2, space="PSUM"))
            ps3 = ph2s.enter_context(tc.tile_pool(name="l2ps3", bufs=2, space="PSUM"))
            drsb = ph2s.enter_context(tc.tile_pool(name="l2dr", bufs=2))
            for ch in range(NCH):
                db0 = ch * CHUNK_DB
                CB = min(CHUNK_DB, NB - db0)
                c0, c1 = db0 * B, (db0 + CB) * B
                JW = CB * B
                g2 = gpool.tile([128, CHUNK_DB * B, FA2], BF16, tag="g2")
                for j in range(JW):
                    nc.gpsimd.indirect_dma_start(
                        out=g2[:, j, :], out_offset=None,
                        in_=h2_tab[:],
                        in_offset=bass.IndirectOffsetOnAxis(
                            ap=src_tab[:, c0 + j:c0 + j + 1], axis=0))
                S = sb.tile([128, CHUNK_DB * B, 128], BF16, tag="S2")
                nc.vector.tensor_tensor(
                    out=S[:, 0:JW, :],
                    in0=dsl[:, c0:c1].unsqueeze(2).to_broadcast([128, JW, 128]),
                    in1=rconst[:].unsqueeze(1).to_broadcast([128, JW, 128]),
                    op=OP.is_equal)
                a2b4 = sb.tile([128, CHUNK_DB, 1], BF16, tag="a2b4")
                nc.sync.dma_start(
                    out=a2b4[:, 0:CB, :],
                    in_=adc2_t[db0 * 128:(db0 + CB) * 128, :]
                    .rearrange("(b p) h -> p b h", p=128))
                a2E = sb.tile([128, CHUNK_DB * B, 1], BF16, tag="a2E")
                for q in range(0, JW, 4):
                    qw = min(4, JW - q)
                    pst = ps3.tile([128, 4, 128], BF16, tag="pst2")
                    for k in range(qw):
                        nc.tensor.transpose(out=pst[:, k, :], in_=S[:, q + k, :],
                                            identity=ident[:])
                    stb = sb.tile([128, 4, 128], BF16, tag="stb2")
                    nc.vector.tensor_copy(out=stb[:, 0:qw, :], in_=pst[:, 0:qw, :])
                    for k in range(qw):
                        pad = ps3.tile([128, 1], F32, tag="pad2")
                        nc.tensor.matmul(pad[:], lhsT=stb[:, k, :],
                                         rhs=a2b4[:, (q + k) // B, :],
                                         start=True, stop=True)
                        nc.vector.tensor_copy(out=a2E[:, q + k, :], in_=pad[:])
                ee = sb.tile([128, CHUNK_DB * B, 1], F32, tag="ee2")
                nc.vector.tensor_tensor(out=ee[:, 0:JW, :],
                                        in0=g2[:, 0:JW, C:FA2],
                                        in1=a2E[:, 0:JW, :], op=OP.add)
                nc.vector.scalar_tensor_tensor(
                    out=ee[:, 0:JW, :], in0=ee[:, 0:JW, :], scalar=0.2,
                    in1=ee[:, 0:JW, :], op0=OP.mult, op1=OP.max)
                pt = sb.tile([128, CHUNK_DB * B, 1], BF16, tag="pt2")
                nc.scalar.activation(pt[:, 0:JW, :], ee[:, 0:JW, :], AF.Exp)
                rhs = sb.tile([128, CHUNK_DB * B, FR2], BF16, tag="rhs2")
                nc.vector.tensor_tensor(
                    out=rhs[:, 0:JW, 0:C],
                    in0=g2[:, 0:JW, 0:C],
                    in1=pt[:, 0:JW, :].to_broadcast([128, JW, C]),
                    op=OP.mult)
                nc.vector.tensor_copy(out=rhs[:, 0:JW, C:FR2], in_=pt[:, 0:JW, :])

                for lb in range(CB):
                    db = db0 + lb
                    acc = ps.tile([128, FR2], F32, tag="acc2")
                    for j in range(B):
                        jj = lb * B + j
                        nc.tensor.matmul(acc[:], lhsT=S[:, jj, :],
                                         rhs=rhs[:, jj, :],
                                         start=(j == 0), stop=(j == B - 1))
                    rec = drsb.tile([128, 1], F32, tag="rec2")
                    nc.vector.tensor_scalar_add(rec[:], acc[:, C:FR2], 1e-16)
                    nc.vector.reciprocal(rec[:], rec[:])
                    o2 = drsb.tile([128, C], F32, tag="o2")
                    nc.vector.tensor_tensor(
                        out=o2[:], in0=acc[:, 0:C],
                        in1=rec[:].to_broadcast([128, C]), op=OP.mult)
                    nc.vector.tensor_add(out=o2[:], in0=o2[:], in1=b2rep[:])
                    # log_softmax
                    mneg = drsb.tile([128, 1], F32, tag="mneg")
                    nc.vector.tensor_reduce(out=mneg[:], in_=o2[:], axis=AX.X,
                                            op=OP.max, negate=True)
                    escr = drsb.tile([128, C], F32, tag="escr")
                    ssum = drsb.tile([128, 1], F32, tag="ssum")
                    nc.scalar.activation(escr[:], o2[:], AF.Exp,
                                         bias=mneg[:, 0:1], accum_out=ssum[:])
                    lns = drsb.tile([128, 1], F32, tag="lns")
                    nc.scalar.activation(lns[:], ssum[:], AF.Ln)
                    tsh = drsb.tile([128, 1], F32, tag="tsh")
                    nc.vector.tensor_sub(out=tsh[:], in0=mneg[:], in1=lns[:])
                    fin = drsb.tile([128, C], F32, tag="fin")
                    nc.vector.tensor_scalar(out=fin[:], in0=o2[:],
                                            scalar1=tsh[:, 0:1], scalar2=None,
                                            op0=OP.add)
                    nv = 128 if db < NB - 1 else LASTV
                    nc.sync.dma_start(out=out_d[db * 128:db * 128 + nv, :],
                                      in_=fin[:nv, :])

    nc.compile()
    return nc


_NC_CACHE: dict = {}


def _get_nc(cfg: Cfg):
    if cfg not in _NC_CACHE:
        _NC_CACHE[cfg] = build_nc(cfg)
    return _NC_CACHE[cfg]


def kernel(x, edge_index, W1, a_src1, a_dst1, b1, W2, a_src2, a_dst2, b2,
           cfg: Cfg | None = None, _run=None):
    x = np.asarray(x)
    edge_index = np.asarray(edge_index)
    if cfg is None:
        cfg = Cfg()
        loops = np.arange(cfg.N, dtype=np.int64)
        dst = np.concatenate([np.asarray(edge_index[1]).astype(np.int64), loops])
        b = compute_B(dst, cfg)
        if b != cfg.B:
            cfg = Cfg(B=b)
    in_maps = build_in_maps(x, edge_index, W1, a_src1, a_dst1, b1,
                            W2, a_src2, a_dst2, b2, cfg)
    nc = _get_nc(cfg)
    if _run is not None:
        results = _run(nc, in_maps)
    else:
        res = run_bass_kernel_spmd(nc, in_maps, list(range(cfg.NC)))
        results = res.results
    out = np.concatenate([results[c]["out"] for c in range(cfg.NC)], axis=0)
    return out.astype(np.float32)

